# revision 2
# baseline (speedup 1.0000x reference)
"""SIREN-generated group-equivariant conv (GroupConv) on 8 trn2 cores.

Strategy (matches the data-parallel hint): the SIREN kernel-generator is
tiny (~0.2 GFLOP) and deterministic, so the conv weight is computed on
host once; the 420-GFLOP 7x7 'same' conv runs on device, batch-sharded
B=16 -> 2 per core. The conv is expressed as 49 shifted matmuls (one per
tap) over a zero-padded input, accumulating in PSUM:
  out[O, p] += W_tap[I, O]^T @ xpad[I, shifted window p]
with I = Cin*Gin = 256 (2 K-tiles of 128) and O = Gout*Cout = 256
(2 M-tiles of 128), spatial tiled as 8 rows x 64 cols = 512 (1 PSUM bank).
"""

import os
import sys

sys.path.insert(0, "/opt/trn_rl_repo")

import numpy as np

KS = 7
OMEGA0 = 10.0
PAD = 3
B, CIN, COUT, G, H, W, HID = 16, 32, 32, 8, 64, 64, 32
NCORES = 8
BLOC = B // NCORES          # batch per core
IC = CIN * G                # 256 input channels to the conv
OC = G * COUT               # 256 output channels of the conv
HP, WP = H + 2 * PAD, W + 2 * PAD  # 70, 70
NKT = IC // 128             # K tiles
NMT = OC // 128             # M tiles
ROWS_PER_TILE = 8           # 8*64 = 512 = one fp32 PSUM bank
NST = H // ROWS_PER_TILE    # spatial tiles

# matmul input mode: "f32" (exact, 4 cyc/row), "f32r" (relaxed fp32,
# 1 cyc/row at N>=256), "bf16" (1 cyc/row, lossy)
MODE = os.environ.get("GC_MODE", "f32r")


def _gen_weight_np(input_g_elems, W1, b1, W2, b2, W3, b3):
    """Mirror of reference.py's kernel generation, numpy float32."""
    f32 = np.float32
    lin = np.linspace(-1.0, 1.0, KS, dtype=f32)
    yy, xx = np.meshgrid(lin, lin, indexing="ij")
    rel_pos = np.stack([yy, xx], axis=0)                       # [2,ks,ks]

    thetas = input_g_elems[0].astype(f32)                      # [G]
    Gout = thetas.shape[0]
    two_pi = f32(2.0 * np.pi)
    acted_g = (np.mod(thetas[None, :] - thetas[:, None], two_pi)
               / f32(np.pi) - f32(1.0)).astype(f32)            # [Gout,Gin]

    ct, st = np.cos(-thetas).astype(f32), np.sin(-thetas).astype(f32)
    R = np.stack([np.stack([ct, -st], -1), np.stack([st, ct], -1)], -2)
    acted_rel = np.einsum("gij,jhw->gihw", R, rel_pos).astype(f32)

    Gin = thetas.shape[0]
    pos_rd = np.broadcast_to(acted_rel[:, :, None], (Gout, 2, Gin, KS, KS))
    pos_g = np.broadcast_to(acted_g[:, None, :, None, None], (Gout, 1, Gin, KS, KS))
    coords = np.concatenate([pos_rd, pos_g], axis=1)
    points = np.moveaxis(coords, 1, -1).astype(f32)            # [Gout,Gin,ks,ks,3]

    h = np.sin(f32(OMEGA0) * (points @ W1 + b1)).astype(f32)
    h = np.sin(f32(OMEGA0) * (h @ W2 + b2)).astype(f32)
    kernel = (h @ W3 + b3).astype(f32)                         # [...,Cout*Cin]

    Cout = kernel.shape[-1] // CIN
    kernel = kernel.reshape(Gout, Gin, KS, KS, Cout, CIN)
    kernel = np.transpose(kernel, (0, 4, 5, 1, 2, 3))          # [Gout,Cout,Cin,Gin,ks,ks]

    mask = (np.sqrt((acted_rel ** 2).sum(axis=1)) <= 1.0)      # [Gout,ks,ks]
    kernel = kernel * mask[:, None, None, None, :, :].astype(f32)
    return kernel.reshape(Gout * Cout, CIN * Gin, KS, KS)      # [O, I, ks, ks]


def _gen_weight_jax(input_g_elems, W1, b1, W2, b2, W3, b3):
    """Same computation via jax on CPU (bit-matches the jax reference)."""
    import jax
    import jax.numpy as jnp

    cpu = jax.local_devices(backend="cpu")[0]
    with jax.default_device(cpu):
        lin = jnp.linspace(-1.0, 1.0, KS, dtype=jnp.float32)
        yy, xx = jnp.meshgrid(lin, lin, indexing="ij")
        rel_pos = jnp.stack([yy, xx], axis=0)

        thetas = jnp.asarray(input_g_elems)[0]
        Gout = thetas.shape[0]
        two_pi = 2.0 * jnp.pi
        acted_g = jnp.mod(thetas[None, :] - thetas[:, None], two_pi) / jnp.pi - 1.0

        ct, st = jnp.cos(-thetas), jnp.sin(-thetas)
        R = jnp.stack([jnp.stack([ct, -st], -1), jnp.stack([st, ct], -1)], -2)
        acted_rel = jnp.einsum("gij,jhw->gihw", R, rel_pos)

        Gin = thetas.shape[0]
        pos_rd = jnp.broadcast_to(acted_rel[:, :, None], (Gout, 2, Gin, KS, KS))
        pos_g = jnp.broadcast_to(acted_g[:, None, :, None, None], (Gout, 1, Gin, KS, KS))
        coords = jnp.concatenate([pos_rd, pos_g], axis=1)
        points = jnp.moveaxis(coords, 1, -1)

        h = jnp.sin(OMEGA0 * (points @ jnp.asarray(W1) + jnp.asarray(b1)))
        h = jnp.sin(OMEGA0 * (h @ jnp.asarray(W2) + jnp.asarray(b2)))
        kernel = h @ jnp.asarray(W3) + jnp.asarray(b3)

        Cout = kernel.shape[-1] // CIN
        kernel = kernel.reshape(Gout, Gin, KS, KS, Cout, CIN)
        kernel = jnp.transpose(kernel, (0, 4, 5, 1, 2, 3))
        mask = jnp.linalg.norm(acted_rel, axis=1) <= 1.0
        kernel = kernel * mask[:, None, None, None, :, :]
        w = kernel.reshape(Gout * Cout, CIN * Gin, KS, KS)
        return np.asarray(w, dtype=np.float32)


def _gen_weight(*args):
    try:
        return _gen_weight_jax(*args)
    except Exception:
        return _gen_weight_np(*args)


_NC_CACHE = {}


def _build_nc(mode):
    import concourse.bass as bass
    import concourse.tile as tile
    from concourse import mybir

    # --- workaround: walrus codegen in this toolchain rejects >1 sem wait
    # per instruction. Hoist extra waits onto injected NoOps on the same
    # engine (body instructions), and split the tail-drain's wait list
    # across several drain instructions. ---
    if not getattr(tile.TileContext, "_waitsplit_patched", False):
        import bass_rust

        _orig_add = tile.TileContext._add_instruction

        def _add_split_waits(self, inst):
            si = getattr(inst, "sync_info", None)
            if si is not None and len(si.on_wait) > 1:
                waits = list(si.on_wait)
                si.on_wait = waits[:1]
                for w in waits[1:]:
                    nop = bass_rust.InstNoOp(
                        name=self.nc.get_next_instruction_name(),
                        ins=[], outs=[], text_hint="waitsplit",
                    )
                    nop.engine = inst.engine
                    nop.sync_info = mybir.SyncInfo(on_wait=[w], on_update=[])
                    _orig_add(self, nop)
            _orig_add(self, inst)

        tile.TileContext._add_instruction = _add_split_waits
        tile.TileContext._waitsplit_patched = True

    if not getattr(tile.TileContext, "_drain_split_patched", False):
        def _split_drain_and_barrier(self, tick_clock, wait_clock):
            nc = self.nc
            drain_inst = nc.sync.drain()
            wait_clock.add_sem_waits(
                drain_inst.ins, tile.ScopedClock({None: tick_clock.global_clock})
            )
            si = drain_inst.ins.sync_info
            if si is not None and len(si.on_wait) > 1:
                waits = list(si.on_wait)
                si.on_wait = waits[:1]
                for i in range(1, len(waits)):
                    d2 = nc.sync.drain()
                    s2 = d2.ins.sync_info
                    if s2 is None:
                        d2.ins.sync_info = mybir.SyncInfo(
                            on_wait=waits[i:i + 1], on_update=[]
                        )
                    else:
                        s2.on_wait = waits[i:i + 1]
            nc.all_engine_barrier()
            assert self.sems is not None
            popped = nc._tile_sem_poison_stack.pop()
            assert popped is self._sem_poison
            nc.clear_and_free_semaphores(list(self.sems.allocated().values()))
            nc.all_engine_barrier()

        tile.TileContext._drain_and_barrier = _split_drain_and_barrier
        tile.TileContext._drain_split_patched = True

    in_dt = {"f32": mybir.dt.float32,
             "f32r": mybir.dt.float32r,
             "bf16": mybir.dt.bfloat16}[mode]
    f32 = mybir.dt.float32

    nc = bass.Bass()
    xp_d = nc.declare_dram_parameter("xp", [BLOC, IC, HP, WP], in_dt, isOutput=False)
    wT_d = nc.declare_dram_parameter("wT", [128, NKT, KS * KS, OC], in_dt, isOutput=False)
    out_d = nc.declare_dram_parameter("out", [BLOC, OC, H, W], f32, isOutput=True)

    taps = [(ky, kx) for ky in range(KS) for kx in range(KS)]

    with tile.TileContext(nc) as tc:
        with (
            tc.tile_pool(name="wpool", bufs=1) as wpool,
            tc.tile_pool(name="xpool", bufs=3) as xpool,
            tc.tile_pool(name="opool", bufs=4) as opool,
            tc.tile_pool(name="psum", bufs=8, space="PSUM") as psum,
        ):
            wt = wpool.tile([128, NKT, KS * KS, OC], in_dt)
            nc.sync.dma_start(wt[:], wT_d[:])

            for b in range(BLOC):
                xts = []
                for kt in range(NKT):
                    xt = xpool.tile([128, HP, WP], in_dt, tag="xt")
                    nc.sync.dma_start(
                        xt[:], xp_d[b, kt * 128:(kt + 1) * 128, :, :]
                    )
                    xts.append(xt)

                for mt in range(NMT):
                    for s in range(NST):
                        ps = psum.tile([128, ROWS_PER_TILE, W], f32)
                        r0 = s * ROWS_PER_TILE
                        n = NKT * len(taps)
                        i = 0
                        for kt in range(NKT):
                            for t, (ky, kx) in enumerate(taps):
                                nc.tensor.matmul(
                                    ps[:],
                                    wt[:, kt, t, mt * 128:(mt + 1) * 128],
                                    xts[kt][:, r0 + ky:r0 + ky + ROWS_PER_TILE,
                                            kx:kx + W],
                                    start=(i == 0),
                                    stop=(i == n - 1),
                                )
                                i += 1
                        ot = opool.tile([128, ROWS_PER_TILE, W], f32, tag="ot")
                        nc.vector.tensor_copy(ot[:], ps[:])
                        nc.sync.dma_start(
                            out_d[b, mt * 128:(mt + 1) * 128,
                                  r0:r0 + ROWS_PER_TILE, :],
                            ot[:],
                        )
    return nc


def _get_nc(mode):
    if mode not in _NC_CACHE:
        _NC_CACHE[mode] = _build_nc(mode)
    return _NC_CACHE[mode]


def kernel(x, input_g_elems, W1, b1, W2, b2, W3, b3, bias,
           _trace=False, _trace_kwargs=None):
    from concourse.bass_utils import run_bass_kernel_spmd

    weight = _gen_weight(input_g_elems, W1, b1, W2, b2, W3, b3)  # [O, I, 7, 7]

    # lhsT layout: [ci(128), kt, tap, O]
    w4 = weight.reshape(OC, NKT, 128, KS, KS)
    lhsT = np.ascontiguousarray(
        np.transpose(w4, (2, 1, 3, 4, 0)).reshape(128, NKT, KS * KS, OC)
    )

    xf = np.asarray(x, dtype=np.float32).reshape(B, IC, H, W)
    xp = np.pad(xf, ((0, 0), (0, 0), (PAD, PAD), (PAD, PAD)))
    shards = xp.reshape(NCORES, BLOC, IC, HP, WP)

    if MODE == "bf16":
        import ml_dtypes
        lhsT = lhsT.astype(ml_dtypes.bfloat16)
        shards = shards.astype(ml_dtypes.bfloat16)

    nc = _get_nc(MODE)
    in_maps = [{"xp": shards[i], "wT": lhsT} for i in range(NCORES)]
    res = run_bass_kernel_spmd(
        nc, in_maps, list(range(NCORES)),
        trace=_trace, **(_trace_kwargs or {})
    )

    out = np.stack([res.results[i]["out"] for i in range(NCORES)])
    out = out.reshape(B, G, COUT, H, W).transpose(0, 2, 1, 3, 4)
    out = out + np.asarray(bias, dtype=np.float32).reshape(1, COUT, 1, 1, 1)
    out = np.ascontiguousarray(out, dtype=np.float32)
    if _trace:
        return out, res
    return out


# revision 6
# speedup vs baseline: 1.0075x; 1.0075x over previous
"""SIREN-generated group-equivariant conv (GroupConv) on 8 trn2 cores.

Strategy (matches the data-parallel hint): the SIREN kernel-generator is
tiny (~0.2 GFLOP) and deterministic, so the conv weight is computed on
host once; the 420-GFLOP 7x7 'same' conv runs on device, batch-sharded
B=16 -> 2 per core. The conv is expressed as 49 shifted matmuls (one per
tap) over a zero-padded input, accumulating in PSUM:
  out[O, p] += W_tap[I, O]^T @ xpad[I, shifted window p]
with I = Cin*Gin = 256 (2 K-tiles of 128) and O = Gout*Cout = 256
(2 M-tiles of 128), spatial tiled as 8 rows x 64 cols = 512 (1 PSUM bank).
"""

import os
import sys

sys.path.insert(0, "/opt/trn_rl_repo")

import numpy as np

KS = 7
OMEGA0 = 10.0
PAD = 3
B, CIN, COUT, G, H, W, HID = 16, 32, 32, 8, 64, 64, 32
NCORES = 8
BLOC = B // NCORES          # batch per core
IC = CIN * G                # 256 input channels to the conv
OC = G * COUT               # 256 output channels of the conv
HP, WP = H + 2 * PAD, W + 2 * PAD  # 70, 70
NKT = IC // 128             # K tiles
NMT = OC // 128             # M tiles
ROWS_PER_TILE = 8           # 8*64 = 512 = one fp32 PSUM bank
NST = H // ROWS_PER_TILE    # spatial tiles

# matmul input mode: "f32" (exact, 4 cyc/row), "f32r" (relaxed fp32,
# 1 cyc/row at N>=256), "bf16" (1 cyc/row, lossy)
MODE = os.environ.get("GC_MODE", "f32r")


def _gen_weight_np(input_g_elems, W1, b1, W2, b2, W3, b3):
    """Mirror of reference.py's kernel generation, numpy float32."""
    f32 = np.float32
    lin = np.linspace(-1.0, 1.0, KS, dtype=f32)
    yy, xx = np.meshgrid(lin, lin, indexing="ij")
    rel_pos = np.stack([yy, xx], axis=0)                       # [2,ks,ks]

    thetas = input_g_elems[0].astype(f32)                      # [G]
    Gout = thetas.shape[0]
    two_pi = f32(2.0 * np.pi)
    acted_g = (np.mod(thetas[None, :] - thetas[:, None], two_pi)
               / f32(np.pi) - f32(1.0)).astype(f32)            # [Gout,Gin]

    ct, st = np.cos(-thetas).astype(f32), np.sin(-thetas).astype(f32)
    R = np.stack([np.stack([ct, -st], -1), np.stack([st, ct], -1)], -2)
    acted_rel = np.einsum("gij,jhw->gihw", R, rel_pos).astype(f32)

    Gin = thetas.shape[0]
    pos_rd = np.broadcast_to(acted_rel[:, :, None], (Gout, 2, Gin, KS, KS))
    pos_g = np.broadcast_to(acted_g[:, None, :, None, None], (Gout, 1, Gin, KS, KS))
    coords = np.concatenate([pos_rd, pos_g], axis=1)
    points = np.moveaxis(coords, 1, -1).astype(f32)            # [Gout,Gin,ks,ks,3]

    h = np.sin(f32(OMEGA0) * (points @ W1 + b1)).astype(f32)
    h = np.sin(f32(OMEGA0) * (h @ W2 + b2)).astype(f32)
    kernel = (h @ W3 + b3).astype(f32)                         # [...,Cout*Cin]

    Cout = kernel.shape[-1] // CIN
    kernel = kernel.reshape(Gout, Gin, KS, KS, Cout, CIN)
    kernel = np.transpose(kernel, (0, 4, 5, 1, 2, 3))          # [Gout,Cout,Cin,Gin,ks,ks]

    mask = (np.sqrt((acted_rel ** 2).sum(axis=1)) <= 1.0)      # [Gout,ks,ks]
    kernel = kernel * mask[:, None, None, None, :, :].astype(f32)
    return kernel.reshape(Gout * Cout, CIN * Gin, KS, KS)      # [O, I, ks, ks]


def _gen_weight_jax(input_g_elems, W1, b1, W2, b2, W3, b3):
    """Same computation via jax on CPU (bit-matches the jax reference)."""
    import jax
    import jax.numpy as jnp

    cpu = jax.local_devices(backend="cpu")[0]
    with jax.default_device(cpu):
        lin = jnp.linspace(-1.0, 1.0, KS, dtype=jnp.float32)
        yy, xx = jnp.meshgrid(lin, lin, indexing="ij")
        rel_pos = jnp.stack([yy, xx], axis=0)

        thetas = jnp.asarray(input_g_elems)[0]
        Gout = thetas.shape[0]
        two_pi = 2.0 * jnp.pi
        acted_g = jnp.mod(thetas[None, :] - thetas[:, None], two_pi) / jnp.pi - 1.0

        ct, st = jnp.cos(-thetas), jnp.sin(-thetas)
        R = jnp.stack([jnp.stack([ct, -st], -1), jnp.stack([st, ct], -1)], -2)
        acted_rel = jnp.einsum("gij,jhw->gihw", R, rel_pos)

        Gin = thetas.shape[0]
        pos_rd = jnp.broadcast_to(acted_rel[:, :, None], (Gout, 2, Gin, KS, KS))
        pos_g = jnp.broadcast_to(acted_g[:, None, :, None, None], (Gout, 1, Gin, KS, KS))
        coords = jnp.concatenate([pos_rd, pos_g], axis=1)
        points = jnp.moveaxis(coords, 1, -1)

        h = jnp.sin(OMEGA0 * (points @ jnp.asarray(W1) + jnp.asarray(b1)))
        h = jnp.sin(OMEGA0 * (h @ jnp.asarray(W2) + jnp.asarray(b2)))
        kernel = h @ jnp.asarray(W3) + jnp.asarray(b3)

        Cout = kernel.shape[-1] // CIN
        kernel = kernel.reshape(Gout, Gin, KS, KS, Cout, CIN)
        kernel = jnp.transpose(kernel, (0, 4, 5, 1, 2, 3))
        mask = jnp.linalg.norm(acted_rel, axis=1) <= 1.0
        kernel = kernel * mask[:, None, None, None, :, :]
        w = kernel.reshape(Gout * Cout, CIN * Gin, KS, KS)
        return np.asarray(w, dtype=np.float32)


def _gen_weight(*args):
    try:
        return _gen_weight_jax(*args)
    except Exception:
        return _gen_weight_np(*args)


_NC_CACHE = {}


def _build_nc(mode):
    import concourse.bass as bass
    import concourse.tile as tile
    from concourse import mybir

    # --- workaround: walrus codegen in this toolchain rejects >1 sem wait
    # per instruction. Hoist extra waits onto injected NoOps on the same
    # engine (body instructions), and split the tail-drain's wait list
    # across several drain instructions. ---
    if not getattr(tile.TileContext, "_waitsplit_patched", False):
        import bass_rust

        _orig_add = tile.TileContext._add_instruction

        def _add_split_waits(self, inst):
            si = getattr(inst, "sync_info", None)
            if si is not None and len(si.on_wait) > 1:
                waits = list(si.on_wait)
                si.on_wait = waits[:1]
                for w in waits[1:]:
                    nop = bass_rust.InstNoOp(
                        name=self.nc.get_next_instruction_name(),
                        ins=[], outs=[], text_hint="waitsplit",
                    )
                    nop.engine = inst.engine
                    nop.sync_info = mybir.SyncInfo(on_wait=[w], on_update=[])
                    _orig_add(self, nop)
            _orig_add(self, inst)

        tile.TileContext._add_instruction = _add_split_waits
        tile.TileContext._waitsplit_patched = True

    if not getattr(tile.TileContext, "_drain_split_patched", False):
        def _split_drain_and_barrier(self, tick_clock, wait_clock):
            nc = self.nc
            drain_inst = nc.sync.drain()
            wait_clock.add_sem_waits(
                drain_inst.ins, tile.ScopedClock({None: tick_clock.global_clock})
            )
            si = drain_inst.ins.sync_info
            if si is not None and len(si.on_wait) > 1:
                waits = list(si.on_wait)
                si.on_wait = waits[:1]
                for i in range(1, len(waits)):
                    d2 = nc.sync.drain()
                    s2 = d2.ins.sync_info
                    if s2 is None:
                        d2.ins.sync_info = mybir.SyncInfo(
                            on_wait=waits[i:i + 1], on_update=[]
                        )
                    else:
                        s2.on_wait = waits[i:i + 1]
            nc.all_engine_barrier()
            assert self.sems is not None
            popped = nc._tile_sem_poison_stack.pop()
            assert popped is self._sem_poison
            nc.clear_and_free_semaphores(list(self.sems.allocated().values()))
            nc.all_engine_barrier()

        tile.TileContext._drain_and_barrier = _split_drain_and_barrier
        tile.TileContext._drain_split_patched = True

    in_dt = {"f32": mybir.dt.float32,
             "f32r": mybir.dt.float32r,
             "bf16": mybir.dt.bfloat16}[mode]
    f32 = mybir.dt.float32

    nc = bass.Bass()
    xp_d = nc.declare_dram_parameter("xp", [BLOC, IC, HP, WP], in_dt, isOutput=False)
    # weight chunks: [mt, kt, ci(128), tap, o(128)] — contiguous per (mt,kt)
    wT_d = nc.declare_dram_parameter(
        "wT", [NMT, NKT, 128, KS * KS, 128], in_dt, isOutput=False)
    out_d = nc.declare_dram_parameter("out", [BLOC, OC, H, W], f32, isOutput=True)

    taps = [(ky, kx) for ky in range(KS) for kx in range(KS)]
    ntap = len(taps)

    with tile.TileContext(nc) as tc:
        with (
            tc.tile_pool(name="wpool", bufs=1) as wpool,
            tc.tile_pool(name="xpool", bufs=3) as xpool,
            tc.tile_pool(name="opool", bufs=4) as opool,
            tc.tile_pool(name="psum", bufs=8, space="PSUM") as psum,
        ):
            wts = {}
            for mt in range(NMT):
                for kt in range(NKT):
                    w = wpool.tile([128, KS * KS, 128], in_dt, tag=f"w{mt}{kt}")
                    nc.sync.dma_start(w[:], wT_d[mt, kt])
                    wts[(mt, kt)] = w

            for b in range(BLOC):
                xts = []
                for kt in range(NKT):
                    xt = xpool.tile([128, HP, WP], in_dt, tag="xt")
                    nc.sync.dma_start(
                        xt[:], xp_d[b, kt * 128:(kt + 1) * 128, :, :]
                    )
                    xts.append(xt)

                for mt in range(NMT):
                    # all 8 spatial tiles accumulate together so each
                    # (kt,tap) weight is stationary for 8 back-to-back MMs
                    pss = []
                    for s in range(NST):
                        ps = psum.tile([128, ROWS_PER_TILE, W], f32,
                                       tag="ps", name=f"ps_{b}_{mt}_{s}")
                        pss.append(ps)
                    for kt in range(NKT):
                        for t, (ky, kx) in enumerate(taps):
                            for s in range(NST):
                                r0 = s * ROWS_PER_TILE
                                nc.tensor.matmul(
                                    pss[s][:],
                                    wts[(mt, kt)][:, t, :],
                                    xts[kt][:, r0 + ky:r0 + ky + ROWS_PER_TILE,
                                            kx:kx + W],
                                    start=(kt == 0 and t == 0),
                                    stop=(kt == NKT - 1 and t == ntap - 1),
                                )
                    for s in range(NST):
                        ot = opool.tile([128, ROWS_PER_TILE, W], f32, tag="ot")
                        nc.vector.tensor_copy(ot[:], pss[s][:])
                        nc.sync.dma_start(
                            out_d[b, mt * 128:(mt + 1) * 128,
                                  s * ROWS_PER_TILE:(s + 1) * ROWS_PER_TILE, :],
                            ot[:],
                        )
    return nc


def _get_nc(mode):
    if mode not in _NC_CACHE:
        _NC_CACHE[mode] = _build_nc(mode)
    return _NC_CACHE[mode]


def _maybe_enable_ldw_opt():
    """With s-innermost matmul order, 8 consecutive MMs share one weight;
    walrus's LDWEIGHTS dedup (off by default in this toolchain) can elide
    7 of 8 loads."""
    if os.environ.get("GC_LDWOPT", "0") != "1":
        return
    import concourse.bass_utils as bu
    if getattr(bu, "_ldwopt_patched", False):
        return
    _orig = bu.get_walrus_args

    def _gwa(*a, **k):
        return [x.replace("--enable-ldw-opt=false", "--enable-ldw-opt=true")
                for x in _orig(*a, **k)]

    bu.get_walrus_args = _gwa
    bu._ldwopt_patched = True


def kernel(x, input_g_elems, W1, b1, W2, b2, W3, b3, bias,
           _trace=False, _trace_kwargs=None):
    from concourse.bass_utils import run_bass_kernel_spmd

    _maybe_enable_ldw_opt()

    weight = _gen_weight(input_g_elems, W1, b1, W2, b2, W3, b3)  # [O, I, 7, 7]

    # lhsT chunks: [mt, kt, ci(128), tap, o(128)]
    w4 = weight.reshape(NMT, 128, NKT, 128, KS * KS)  # [mt, o, kt, ci, tap]
    lhsT = np.ascontiguousarray(np.transpose(w4, (0, 2, 3, 4, 1)))

    xf = np.asarray(x, dtype=np.float32).reshape(B, IC, H, W)
    xp = np.pad(xf, ((0, 0), (0, 0), (PAD, PAD), (PAD, PAD)))
    shards = xp.reshape(NCORES, BLOC, IC, HP, WP)

    if MODE == "bf16":
        import ml_dtypes
        lhsT = lhsT.astype(ml_dtypes.bfloat16)
        shards = shards.astype(ml_dtypes.bfloat16)

    nc = _get_nc(MODE)
    in_maps = [{"xp": shards[i], "wT": lhsT} for i in range(NCORES)]
    res = run_bass_kernel_spmd(
        nc, in_maps, list(range(NCORES)),
        trace=_trace, **(_trace_kwargs or {})
    )

    out = np.stack([res.results[i]["out"] for i in range(NCORES)])
    out = out.reshape(B, G, COUT, H, W).transpose(0, 2, 1, 3, 4)
    out = out + np.asarray(bias, dtype=np.float32).reshape(1, COUT, 1, 1, 1)
    out = np.ascontiguousarray(out, dtype=np.float32)
    if _trace:
        return out, res
    return out


# revision 8
# speedup vs baseline: 1.0266x; 1.0189x over previous
"""SIREN-generated group-equivariant conv (GroupConv) on 8 trn2 cores.

Strategy (matches the data-parallel hint): the SIREN kernel-generator is
tiny (~0.2 GFLOP) and deterministic, so the conv weight is computed on
host once; the 420-GFLOP 7x7 'same' conv runs on device, batch-sharded
B=16 -> 2 per core. The conv is expressed as 49 shifted matmuls (one per
tap) over a zero-padded input, accumulating in PSUM:
  out[O, p] += W_tap[I, O]^T @ xpad[I, shifted window p]
with I = Cin*Gin = 256 (2 K-tiles of 128) and O = Gout*Cout = 256
(2 M-tiles of 128), spatial tiled as 8 rows x 64 cols = 512 (1 PSUM bank).
"""

import os
import sys

sys.path.insert(0, "/opt/trn_rl_repo")

import numpy as np

KS = 7
OMEGA0 = 10.0
PAD = 3
B, CIN, COUT, G, H, W, HID = 16, 32, 32, 8, 64, 64, 32
NCORES = 8
BLOC = B // NCORES          # batch per core
IC = CIN * G                # 256 input channels to the conv
OC = G * COUT               # 256 output channels of the conv
HP, WP = H + 2 * PAD, W + 2 * PAD  # 70, 70
NKT = IC // 128             # K tiles
NMT = OC // 128             # M tiles
ROWS_PER_TILE = 8           # 8*64 = 512 = one fp32 PSUM bank
NST = H // ROWS_PER_TILE    # spatial tiles

# matmul input mode: "f32" (exact, 4 cyc/row), "f32r" (relaxed fp32,
# 1 cyc/row at N>=256), "bf16" (1 cyc/row, lossy)
MODE = os.environ.get("GC_MODE", "f32r")


def _gen_weight_np(input_g_elems, W1, b1, W2, b2, W3, b3):
    """Mirror of reference.py's kernel generation, numpy float32."""
    f32 = np.float32
    lin = np.linspace(-1.0, 1.0, KS, dtype=f32)
    yy, xx = np.meshgrid(lin, lin, indexing="ij")
    rel_pos = np.stack([yy, xx], axis=0)                       # [2,ks,ks]

    thetas = input_g_elems[0].astype(f32)                      # [G]
    Gout = thetas.shape[0]
    two_pi = f32(2.0 * np.pi)
    acted_g = (np.mod(thetas[None, :] - thetas[:, None], two_pi)
               / f32(np.pi) - f32(1.0)).astype(f32)            # [Gout,Gin]

    ct, st = np.cos(-thetas).astype(f32), np.sin(-thetas).astype(f32)
    R = np.stack([np.stack([ct, -st], -1), np.stack([st, ct], -1)], -2)
    acted_rel = np.einsum("gij,jhw->gihw", R, rel_pos).astype(f32)

    Gin = thetas.shape[0]
    pos_rd = np.broadcast_to(acted_rel[:, :, None], (Gout, 2, Gin, KS, KS))
    pos_g = np.broadcast_to(acted_g[:, None, :, None, None], (Gout, 1, Gin, KS, KS))
    coords = np.concatenate([pos_rd, pos_g], axis=1)
    points = np.moveaxis(coords, 1, -1).astype(f32)            # [Gout,Gin,ks,ks,3]

    h = np.sin(f32(OMEGA0) * (points @ W1 + b1)).astype(f32)
    h = np.sin(f32(OMEGA0) * (h @ W2 + b2)).astype(f32)
    kernel = (h @ W3 + b3).astype(f32)                         # [...,Cout*Cin]

    Cout = kernel.shape[-1] // CIN
    kernel = kernel.reshape(Gout, Gin, KS, KS, Cout, CIN)
    kernel = np.transpose(kernel, (0, 4, 5, 1, 2, 3))          # [Gout,Cout,Cin,Gin,ks,ks]

    mask = (np.sqrt((acted_rel ** 2).sum(axis=1)) <= 1.0)      # [Gout,ks,ks]
    kernel = kernel * mask[:, None, None, None, :, :].astype(f32)
    return kernel.reshape(Gout * Cout, CIN * Gin, KS, KS)      # [O, I, ks, ks]


def _gen_weight_jax(input_g_elems, W1, b1, W2, b2, W3, b3):
    """Same computation via jax on CPU (bit-matches the jax reference)."""
    import jax
    import jax.numpy as jnp

    cpu = jax.local_devices(backend="cpu")[0]
    with jax.default_device(cpu):
        lin = jnp.linspace(-1.0, 1.0, KS, dtype=jnp.float32)
        yy, xx = jnp.meshgrid(lin, lin, indexing="ij")
        rel_pos = jnp.stack([yy, xx], axis=0)

        thetas = jnp.asarray(input_g_elems)[0]
        Gout = thetas.shape[0]
        two_pi = 2.0 * jnp.pi
        acted_g = jnp.mod(thetas[None, :] - thetas[:, None], two_pi) / jnp.pi - 1.0

        ct, st = jnp.cos(-thetas), jnp.sin(-thetas)
        R = jnp.stack([jnp.stack([ct, -st], -1), jnp.stack([st, ct], -1)], -2)
        acted_rel = jnp.einsum("gij,jhw->gihw", R, rel_pos)

        Gin = thetas.shape[0]
        pos_rd = jnp.broadcast_to(acted_rel[:, :, None], (Gout, 2, Gin, KS, KS))
        pos_g = jnp.broadcast_to(acted_g[:, None, :, None, None], (Gout, 1, Gin, KS, KS))
        coords = jnp.concatenate([pos_rd, pos_g], axis=1)
        points = jnp.moveaxis(coords, 1, -1)

        h = jnp.sin(OMEGA0 * (points @ jnp.asarray(W1) + jnp.asarray(b1)))
        h = jnp.sin(OMEGA0 * (h @ jnp.asarray(W2) + jnp.asarray(b2)))
        kernel = h @ jnp.asarray(W3) + jnp.asarray(b3)

        Cout = kernel.shape[-1] // CIN
        kernel = kernel.reshape(Gout, Gin, KS, KS, Cout, CIN)
        kernel = jnp.transpose(kernel, (0, 4, 5, 1, 2, 3))
        mask = jnp.linalg.norm(acted_rel, axis=1) <= 1.0
        kernel = kernel * mask[:, None, None, None, :, :]
        w = kernel.reshape(Gout * Cout, CIN * Gin, KS, KS)
        return np.asarray(w, dtype=np.float32)


def _gen_weight(*args):
    try:
        return _gen_weight_jax(*args)
    except Exception:
        return _gen_weight_np(*args)


_NC_CACHE = {}


def _build_nc(mode):
    import concourse.bass as bass
    import concourse.tile as tile
    from concourse import mybir

    # --- workaround: walrus codegen in this toolchain rejects >1 sem wait
    # per instruction. Hoist extra waits onto injected NoOps on the same
    # engine (body instructions), and split the tail-drain's wait list
    # across several drain instructions. ---
    if not getattr(tile.TileContext, "_waitsplit_patched", False):
        import bass_rust

        _orig_add = tile.TileContext._add_instruction

        def _add_split_waits(self, inst):
            si = getattr(inst, "sync_info", None)
            if si is not None and len(si.on_wait) > 1:
                waits = list(si.on_wait)
                si.on_wait = waits[:1]
                for w in waits[1:]:
                    nop = bass_rust.InstNoOp(
                        name=self.nc.get_next_instruction_name(),
                        ins=[], outs=[], text_hint="waitsplit",
                    )
                    nop.engine = inst.engine
                    nop.sync_info = mybir.SyncInfo(on_wait=[w], on_update=[])
                    _orig_add(self, nop)
            _orig_add(self, inst)

        tile.TileContext._add_instruction = _add_split_waits
        tile.TileContext._waitsplit_patched = True

    if not getattr(tile.TileContext, "_drain_split_patched", False):
        def _split_drain_and_barrier(self, tick_clock, wait_clock):
            nc = self.nc
            drain_inst = nc.sync.drain()
            wait_clock.add_sem_waits(
                drain_inst.ins, tile.ScopedClock({None: tick_clock.global_clock})
            )
            si = drain_inst.ins.sync_info
            if si is not None and len(si.on_wait) > 1:
                waits = list(si.on_wait)
                si.on_wait = waits[:1]
                for i in range(1, len(waits)):
                    d2 = nc.sync.drain()
                    s2 = d2.ins.sync_info
                    if s2 is None:
                        d2.ins.sync_info = mybir.SyncInfo(
                            on_wait=waits[i:i + 1], on_update=[]
                        )
                    else:
                        s2.on_wait = waits[i:i + 1]
            nc.all_engine_barrier()
            assert self.sems is not None
            popped = nc._tile_sem_poison_stack.pop()
            assert popped is self._sem_poison
            nc.clear_and_free_semaphores(list(self.sems.allocated().values()))
            nc.all_engine_barrier()

        tile.TileContext._drain_and_barrier = _split_drain_and_barrier
        tile.TileContext._drain_split_patched = True

    in_dt = {"f32": mybir.dt.float32,
             "f32r": mybir.dt.float32r,
             "bf16": mybir.dt.bfloat16}[mode]
    f32 = mybir.dt.float32

    nc = bass.Bass()
    xp_d = nc.declare_dram_parameter("xp", [BLOC, IC, HP, WP], in_dt, isOutput=False)
    # weight chunks: [mt, kt, ci(128), tap, o(128)] — contiguous per (mt,kt)
    wT_d = nc.declare_dram_parameter(
        "wT", [NMT, NKT, 128, KS * KS, 128], in_dt, isOutput=False)
    out_d = nc.declare_dram_parameter("out", [BLOC, OC, H, W], f32, isOutput=True)

    taps = [(ky, kx) for ky in range(KS) for kx in range(KS)]
    ntap = len(taps)

    with tile.TileContext(nc) as tc:
        with (
            tc.tile_pool(name="wpool", bufs=1) as wpool,
            tc.tile_pool(name="xpool", bufs=3) as xpool,
            tc.tile_pool(name="opool", bufs=4) as opool,
            tc.tile_pool(name="psum", bufs=8, space="PSUM") as psum,
        ):
            wts = {}

            def _wt(mt, kt):
                # emit each weight-chunk DMA at first use: scheduler
                # priority then defers non-critical chunks behind the
                # first group's critical-path loads
                if (mt, kt) not in wts:
                    w = wpool.tile([128, KS * KS, 128], in_dt,
                                   tag=f"w{mt}{kt}", name=f"w_{mt}_{kt}")
                    nc.sync.dma_start(w[:], wT_d[mt, kt])
                    wts[(mt, kt)] = w
                return wts[(mt, kt)]

            for b in range(BLOC):
                xts = []
                for kt in range(NKT):
                    xt = xpool.tile([128, HP, WP], in_dt, tag="xt")
                    nc.sync.dma_start(
                        xt[:], xp_d[b, kt * 128:(kt + 1) * 128, :, :]
                    )
                    xts.append(xt)

                for mt in range(NMT):
                    # all 8 spatial tiles accumulate together so each
                    # (kt,tap) weight is stationary for 8 back-to-back MMs
                    pss = []
                    for s in range(NST):
                        ps = psum.tile([128, ROWS_PER_TILE, W], f32,
                                       tag="ps", name=f"ps_{b}_{mt}_{s}")
                        pss.append(ps)
                    for kt in range(NKT):
                        for t, (ky, kx) in enumerate(taps):
                            for s in range(NST):
                                r0 = s * ROWS_PER_TILE
                                nc.tensor.matmul(
                                    pss[s][:],
                                    _wt(mt, kt)[:, t, :],
                                    xts[kt][:, r0 + ky:r0 + ky + ROWS_PER_TILE,
                                            kx:kx + W],
                                    start=(kt == 0 and t == 0),
                                    stop=(kt == NKT - 1 and t == ntap - 1),
                                )
                    for s in range(NST):
                        ot = opool.tile([128, ROWS_PER_TILE, W], f32, tag="ot")
                        nc.vector.tensor_copy(ot[:], pss[s][:])
                        nc.sync.dma_start(
                            out_d[b, mt * 128:(mt + 1) * 128,
                                  s * ROWS_PER_TILE:(s + 1) * ROWS_PER_TILE, :],
                            ot[:],
                        )
    return nc


def _get_nc(mode):
    if mode not in _NC_CACHE:
        _NC_CACHE[mode] = _build_nc(mode)
    return _NC_CACHE[mode]


def _maybe_enable_ldw_opt():
    """With s-innermost matmul order, 8 consecutive MMs share one weight;
    walrus's LDWEIGHTS dedup (off by default in this toolchain) can elide
    7 of 8 loads."""
    if os.environ.get("GC_LDWOPT", "0") != "1":
        return
    import concourse.bass_utils as bu
    if getattr(bu, "_ldwopt_patched", False):
        return
    _orig = bu.get_walrus_args

    def _gwa(*a, **k):
        return [x.replace("--enable-ldw-opt=false", "--enable-ldw-opt=true")
                for x in _orig(*a, **k)]

    bu.get_walrus_args = _gwa
    bu._ldwopt_patched = True


def kernel(x, input_g_elems, W1, b1, W2, b2, W3, b3, bias,
           _trace=False, _trace_kwargs=None):
    from concourse.bass_utils import run_bass_kernel_spmd

    _maybe_enable_ldw_opt()

    weight = _gen_weight(input_g_elems, W1, b1, W2, b2, W3, b3)  # [O, I, 7, 7]

    # lhsT chunks: [mt, kt, ci(128), tap, o(128)]
    w4 = weight.reshape(NMT, 128, NKT, 128, KS * KS)  # [mt, o, kt, ci, tap]
    lhsT = np.ascontiguousarray(np.transpose(w4, (0, 2, 3, 4, 1)))

    xf = np.asarray(x, dtype=np.float32).reshape(B, IC, H, W)
    xp = np.pad(xf, ((0, 0), (0, 0), (PAD, PAD), (PAD, PAD)))
    shards = xp.reshape(NCORES, BLOC, IC, HP, WP)

    if MODE == "bf16":
        import ml_dtypes
        lhsT = lhsT.astype(ml_dtypes.bfloat16)
        shards = shards.astype(ml_dtypes.bfloat16)

    nc = _get_nc(MODE)
    in_maps = [{"xp": shards[i], "wT": lhsT} for i in range(NCORES)]
    res = run_bass_kernel_spmd(
        nc, in_maps, list(range(NCORES)),
        trace=_trace, **(_trace_kwargs or {})
    )

    out = np.stack([res.results[i]["out"] for i in range(NCORES)])
    out = out.reshape(B, G, COUT, H, W).transpose(0, 2, 1, 3, 4)
    out = out + np.asarray(bias, dtype=np.float32).reshape(1, COUT, 1, 1, 1)
    out = np.ascontiguousarray(out, dtype=np.float32)
    if _trace:
        return out, res
    return out
